# revision 1
# baseline (speedup 1.0000x reference)
"""Trainium2 Bass kernel: single transformer decoder block
(B=4, T=1024, C=1024, H=16 heads, head_dim=64, FFN hidden=4096).

Sharding: sequence-parallel across 8 NeuronCores = 4 batches x 2
causally-balanced token halves.  No collectives: each core computes
LN1 + K/V projections for all 1024 tokens of its batch, and
attention + FFN for its own 512 tokens.  The host shards/permutes on
the way in and gathers/unpermutes on the way out.

On-chip everything runs in a transposed layout [C on partitions,
tokens on free dim]:
  - LN mean/var via ones-matmul partition reductions (bf16 operands,
    fp32 PSUM) + K=1 broadcast matmuls.
  - Scores are computed transposed (S^T = K_h^T-stationary x Q_h^T);
    causal masking is a multiplicative bf16 0/1 mask on exp(S^T); the
    softmax denominator comes from a ones-matmul accumulated alongside
    the A@V matmul.  No PE transposes, no max-subtraction (logits are
    bounded ~|0.8|).
  - Matmuls in bf16 with fp32 PSUM accumulation (rel err ~2e-3).
  - Every PSUM accumulation chain owns a whole bank (start=True resets
    accumulation state bank-wide, verified on HW).

Token permutation makes the kernel uniform across cores: own token
tiles (sorted by descending causal span) sit at positions 0..3,
partner tiles at 4..7, so even/odd cores share one instruction
stream; per-core causal differences live in the mask01 tensor.

Single long-lived SBUF/PSUM pools with tag-chained slot reuse: no
mid-kernel pool releases (each release is an all-engine barrier).
"""

import numpy as np
import ml_dtypes

B, T, C, H = 4, 1024, 1024, 16
HD = 64
FF = 4096
P = 128
NT = 8  # token tiles == C tiles
OWN = 512
N_CORES = 8
SCALE = 1.0 / 32.0  # 1/sqrt(C)

# own q-tiles per parity, sorted by descending causal span
OWN_TILES = {0: [7, 4, 3, 0], 1: [6, 5, 2, 1]}
# active slot count per s-position p (p<4: own tiles, p>=4: partner tiles)
NP_ACT = [1, 2, 3, 4, 1, 2, 3, 4]

_CACHE = {}


def _perm(par):
    tiles = OWN_TILES[par] + OWN_TILES[1 - par]
    return np.concatenate([np.arange(k * P, (k + 1) * P) for k in tiles])


def _mask01_for(par):
    m = np.ones((P, NT, P), np.float32)
    tri = (np.arange(P)[:, None] <= np.arange(P)[None, :]).astype(np.float32)
    for p in range(4):
        m[:, p, :] = tri
    for j in range(4):
        if OWN_TILES[1 - par][j] > OWN_TILES[par][j]:
            m[:, 4 + j, :] = 0.0
    return m.astype(ml_dtypes.bfloat16)


def _build_nc():
    import concourse.bass as bass
    from concourse import bacc
    import concourse.mybir as mybir
    import concourse.tile as tile

    AF = mybir.ActivationFunctionType
    OP = mybir.AluOpType
    F32 = mybir.dt.float32
    BF = mybir.dt.bfloat16
    ts = bass.ts

    nc = bacc.Bacc()
    d_xT = nc.dram_tensor("xT", [C, T], F32, kind="ExternalInput")
    # blocked weights: see kernel() for the host-side layouts
    d_wq = nc.dram_tensor("wq", [NT, P, C], BF, kind="ExternalInput")
    d_wk = nc.dram_tensor("wk", [NT, P, C], BF, kind="ExternalInput")
    d_wv = nc.dram_tensor("wv", [C, C], BF, kind="ExternalInput")
    d_w1 = nc.dram_tensor("w1", [32, P, C], BF, kind="ExternalInput")
    d_w2 = nc.dram_tensor("w2", [32, P, C], BF, kind="ExternalInput")
    d_bq = nc.dram_tensor("bq", [P, NT], F32, kind="ExternalInput")
    d_bk = nc.dram_tensor("bk", [P, NT], F32, kind="ExternalInput")
    d_bvb = nc.dram_tensor("bvb", [P, C], F32, kind="ExternalInput")
    d_g1 = nc.dram_tensor("g1", [P, NT], F32, kind="ExternalInput")
    d_b1 = nc.dram_tensor("b1", [P, NT], F32, kind="ExternalInput")
    d_g2 = nc.dram_tensor("g2", [P, NT], F32, kind="ExternalInput")
    d_b2 = nc.dram_tensor("b2", [P, NT], F32, kind="ExternalInput")
    d_gf = nc.dram_tensor("gf", [P, NT], F32, kind="ExternalInput")
    d_bf = nc.dram_tensor("bf", [P, NT], F32, kind="ExternalInput")
    d_b1f = nc.dram_tensor("b1f", [P, 32], F32, kind="ExternalInput")
    d_b2f = nc.dram_tensor("b2f", [P, NT], F32, kind="ExternalInput")
    d_mask = nc.dram_tensor("mask", [P, NT, P], BF, kind="ExternalInput")
    d_out = nc.dram_tensor("outT", [C, OWN], F32, kind="ExternalOutput")

    with tile.TileContext(nc) as tc:
        const = tc.alloc_tile_pool(name="const", bufs=1)
        perst = tc.alloc_tile_pool(name="perst", bufs=1)
        work = tc.alloc_tile_pool(name="work", bufs=1)
        ps = tc.alloc_tile_pool(name="ps", bufs=8, space="PSUM")

        ones_bf = const.tile([P, P], BF)
        nc.vector.memset(ones_bf[:], 1.0)
        eps1 = const.tile([1, 1], F32)
        nc.vector.memset(eps1[:], 1e-5)

        # HAM warm-up: ~3.5us of dummy matmuls while input DMAs land
        warm = ps.tile([1, P], F32, tag="bank", name="warm")
        for _ in range(32):
            nc.tensor.matmul(warm[:], ones_bf[:, 0:1], ones_bf[:, :],
                             start=True, stop=True)

        def load_const(name, dram, shape, dt=F32):
            t = const.tile(shape, dt, name=name)
            nc.sync.dma_start(out=t[:], in_=dram[:])
            return t

        xt = work.tile([P, NT, T], F32, tag="A32", name="xt")
        sb_wv = work.tile([P, NT, C], BF, tag="B16", name="sb_wv")
        for c in range(NT):
            nc.sync.dma_start(out=xt[:, c, :], in_=d_xT[ts(c, P), :])
        for c in range(NT):
            nc.sync.dma_start(out=sb_wv[:, c, :], in_=d_wv[ts(c, P), :])

        sb_bq = load_const("sb_bq", d_bq, [P, NT])
        sb_bk = load_const("sb_bk", d_bk, [P, NT])
        sb_bvb = load_const("sb_bvb", d_bvb, [P, C])
        sb_g1 = load_const("sb_g1", d_g1, [P, NT])
        sb_b1 = load_const("sb_b1", d_b1, [P, NT])
        sb_g2 = load_const("sb_g2", d_g2, [P, NT])
        sb_b2 = load_const("sb_b2", d_b2, [P, NT])
        sb_gf = load_const("sb_gf", d_gf, [P, NT])
        sb_bf = load_const("sb_bf", d_bf, [P, NT])
        sb_b1f = load_const("sb_b1f", d_b1f, [P, 32])
        sb_b2f = load_const("sb_b2f", d_b2f, [P, NT])
        sb_mask = load_const("sb_mask", d_mask, [P, NT, P], BF)

        # persistent activations
        hbf = perst.tile([P, NT, T], BF)
        kT = perst.tile([P, NT, T], BF)
        vT = perst.tile([P, NT, C], BF)
        qT = perst.tile([P, NT, OWN], BF)
        x3 = perst.tile([P, NT, OWN], F32)

        def ln_stat(src_ap, cols, sum_ps, sq_ps, first, last, pfx):
            xb = work.tile([P, 512], BF, tag="xb", bufs=2, name=f"{pfx}xb")[:, 0:cols]
            sq = work.tile([P, 512], BF, tag="sq", bufs=2, name=f"{pfx}sqt")[:, 0:cols]
            nc.scalar.activation(xb[:], src_ap, AF.Copy)
            nc.vector.tensor_mul(sq[:], src_ap, src_ap)
            nc.tensor.matmul(sum_ps[:], ones_bf[:, 0:1], xb[:], start=first, stop=last)
            nc.tensor.matmul(sq_ps[:], ones_bf[:, 0:1], sq[:], start=first, stop=last)

        def ln_finish(sum_ps, sq_ps, src, cols, g_ap, b_ap, outs, pfx):
            mu = work.tile([1, 512], F32, tag="mu", bufs=2, name=f"{pfx}mu")[:, 0:cols]
            t2 = work.tile([1, 512], F32, tag="t2", bufs=2, name=f"{pfx}t2")[:, 0:cols]
            rstd = work.tile([1, 512], F32, tag="rstd", bufs=2, name=f"{pfx}rstd")[:, 0:cols]
            nc.scalar.activation(mu[:], sum_ps[:], AF.Copy, scale=1.0 / C)
            nc.scalar.activation(t2[:], sq_ps[:], AF.Copy, scale=1.0 / C)
            nc.vector.tensor_mul(rstd[:], mu[:], mu[:])
            nc.vector.tensor_sub(t2[:], t2[:], rstd[:])
            nc.scalar.activation(t2[:], t2[:], AF.Sqrt, bias=eps1[:])
            nc.vector.reciprocal_approx_fast(out=rstd[:], in_=t2[:])
            mu_bf = work.tile([1, 512], BF, tag="mubf", bufs=2, name=f"{pfx}mubf")[:, 0:cols]
            rs_bf = work.tile([1, 512], BF, tag="rsbf", bufs=2, name=f"{pfx}rsbf")[:, 0:cols]
            nc.scalar.activation(mu_bf[:], mu[:], AF.Copy)
            nc.scalar.activation(rs_bf[:], rstd[:], AF.Copy)
            mu_bc = ps.tile([P, cols], F32, tag="bank", name=f"{pfx}mubc")
            rs_bc = ps.tile([P, cols], F32, tag="bank", name=f"{pfx}rsbc")
            nc.tensor.matmul(mu_bc[:], ones_bf[0:1, 0:P], mu_bf[0:1, :],
                             start=True, stop=True)
            nc.tensor.matmul(rs_bc[:], ones_bf[0:1, 0:P], rs_bf[0:1, :],
                             start=True, stop=True)
            for c in range(NT):
                s = src(c)
                t0 = work.tile([P, 512], F32, tag="t0", bufs=2, name=f"{pfx}t0")[:, 0:cols]
                nc.vector.tensor_sub(t0[:], s, mu_bc[:])
                nc.vector.tensor_mul(t0[:], t0[:], rs_bc[:])
                for dst in outs(c):
                    nc.vector.tensor_scalar(
                        out=dst, in0=t0[:],
                        scalar1=g_ap(c), scalar2=b_ap(c),
                        op0=OP.mult, op1=OP.add)

        def layernorm_T(src, cols, g_ap, b_ap, outs, pfx):
            sum_ps = ps.tile([1, cols], F32, tag="bank", name=f"{pfx}sum")
            sq_ps = ps.tile([1, cols], F32, tag="bank", name=f"{pfx}sq")
            for c in range(NT):
                ln_stat(src(c), cols, sum_ps, sq_ps, c == 0, c == NT - 1, pfx)
            ln_finish(sum_ps, sq_ps, src, cols, g_ap, b_ap, outs, pfx)

        # ---------------- Phase 1: LN1 (split halves) + QKV projections
        # lo half (own tokens) first so q/v projections can start early
        for h2 in range(2):
            cs = slice(h2 * 512, (h2 + 1) * 512)
            layernorm_T(lambda c: xt[:, c, cs], 512,
                        lambda c: sb_g1[:, c:c + 1], lambda c: sb_b1[:, c:c + 1],
                        lambda c: [hbf[:, c, cs]], pfx=f"ln1{h2}")
            # v token-major for this half's token tiles
            for t in range(h2 * 4, h2 * 4 + 4):
                for vh in range(2):
                    v_ps = ps.tile([P, 512], F32, tag="bank", name="v_ps")
                    for c in range(NT):
                        nc.tensor.matmul(v_ps[:], hbf[:, c, ts(t, P)],
                                         sb_wv[:, c, ts(vh, 512)],
                                         start=(c == 0), stop=(c == NT - 1))
                    nc.vector.tensor_add(vT[:, t, ts(vh, 512)], v_ps[:],
                                         sb_bvb[:, ts(vh, 512)])
            if h2 == 0:
                # qT[d, own tokens]
                for hp in range(NT):
                    wq_t = work.tile([P, C], BF, tag="W", bufs=5, name="wq_t")
                    nc.sync.dma_start(out=wq_t[:], in_=d_wq[hp])
                    q_ps = ps.tile([P, OWN], F32, tag="bank", name="q_ps")
                    for c in range(NT):
                        nc.tensor.matmul(q_ps[:], wq_t[:, ts(c, P)], hbf[:, c, 0:OWN],
                                         start=(c == 0), stop=(c == NT - 1))
                    nc.vector.tensor_scalar(out=qT[:, hp, :], in0=q_ps[:],
                                            scalar1=sb_bq[:, hp:hp + 1], scalar2=None,
                                            op0=OP.add)
        # kT[d, all tokens] (overlaps attention start)
        for hp in range(NT):
            wk_t = work.tile([P, C], BF, tag="W", bufs=5, name="wk_t")
            nc.sync.dma_start(out=wk_t[:], in_=d_wk[hp])
            for h2 in range(2):
                k_ps = ps.tile([P, 512], F32, tag="bank", name="k_ps")
                for c in range(NT):
                    nc.tensor.matmul(k_ps[:], wk_t[:, ts(c, P)],
                                     hbf[:, c, ts(h2, 512)],
                                     start=(c == 0), stop=(c == NT - 1))
                nc.vector.tensor_scalar(out=kT[:, hp, ts(h2, 512)], in0=k_ps[:],
                                        scalar1=sb_bk[:, hp:hp + 1], scalar2=None,
                                        op0=OP.add)

        # ---------------- Phase 2: attention ----------------
        x2 = work.tile([P, NT, OWN], F32, tag="A32", name="x2")
        CUM = [0, 128, 384, 768, 1280, 1408, 1664, 2048]  # packed exp offsets
        for hp in range(NT):
            ex_e = work.tile([P, 2560], BF, tag="exe", bufs=2, name="ex_e")
            ex_o = work.tile([P, 2560], BF, tag="exo", bufs=2, name="ex_o")
            for p in range(NT):
                w = NP_ACT[p] * P
                po = CUM[p]
                st_e = ps.tile([P, OWN], F32, tag="bank", name="st_e")
                st_o = ps.tile([P, OWN], F32, tag="bank", name="st_o")
                nc.tensor.matmul(st_e[:, 0:w], kT[0:64, hp, ts(p, P)],
                                 qT[0:64, hp, 0:w], start=True, stop=True)
                nc.tensor.matmul(st_o[:, 0:w], kT[64:128, hp, ts(p, P)],
                                 qT[64:128, hp, 0:w], start=True, stop=True)
                nc.scalar.activation(ex_e[:, po:po + w], st_e[:, 0:w], AF.Exp,
                                     scale=SCALE)
                nc.scalar.activation(ex_o[:, po:po + w], st_o[:, 0:w], AF.Exp,
                                     scale=SCALE)
                im = p % 4
                mw = slice(po + im * P, po + (im + 1) * P)
                nc.vector.tensor_mul(ex_e[:, mw], ex_e[:, mw], sb_mask[:, p, :])
                nc.vector.tensor_mul(ex_o[:, mw], ex_o[:, mw], sb_mask[:, p, :])
            gat = work.tile([P, OWN], F32, tag="gat", bufs=1, name="gat")
            rs = work.tile([P, OWN], F32, tag="rs", bufs=2, name="rs")
            # one PSUM bank per (head, quantity): the p=3 block covers all
            # four slots full-width with start=True, later blocks accumulate
            # per-element on sub-ranges
            oTe = ps.tile([64, OWN], F32, tag="bank", name="oTe")
            oTo = ps.tile([P, OWN], F32, tag="bank", name="oTo")
            seE = ps.tile([64, OWN], F32, tag="bank", name="seE")
            seO = ps.tile([P, OWN], F32, tag="bank", name="seO")
            for p in [3, 0, 1, 2, 4, 5, 6, 7]:
                w = NP_ACT[p] * P
                first = (p == 3)
                last = (p == NT - 1)
                pc = slice(CUM[p], CUM[p] + w)
                nc.tensor.matmul(oTe[:, 0:w], vT[:, p, hp * P:hp * P + 64],
                                 ex_e[:, pc], start=first, stop=last)
                nc.tensor.matmul(oTo[64:128, 0:w],
                                 vT[:, p, hp * P + 64:hp * P + 128],
                                 ex_o[:, pc], start=first, stop=last)
                nc.tensor.matmul(seE[:, 0:w], ones_bf[:, 0:64],
                                 ex_e[:, pc], start=first, stop=last)
                nc.tensor.matmul(seO[64:128, 0:w], ones_bf[:, 0:64],
                                 ex_o[:, pc], start=first, stop=last)
            nc.vector.tensor_copy(gat[0:64, :], seE[:, :])
            nc.vector.tensor_copy(gat[64:128, :], seO[64:128, :])
            nc.vector.reciprocal_approx_fast(out=rs[:], in_=gat[:])
            ot = work.tile([P, OWN], F32, tag="ot", bufs=2, name="ot")
            nc.vector.tensor_mul(ot[0:64, :], oTe[:, :], rs[0:64, :])
            nc.vector.tensor_mul(ot[64:128, :], oTo[64:128, :], rs[64:128, :])
            # x2 = h + attn_out (residual uses post-LN h)
            nc.vector.tensor_add(x2[:, hp, :], ot[:, :], hbf[:, hp, 0:OWN])

        # ---------------- Phase 3: LN2, LNf, FFN ----------------
        hfb = work.tile([P, NT, OWN], BF, tag="C16", name="hfb")
        layernorm_T(lambda c: x2[:, c, :], 512,
                    lambda c: sb_g2[:, c:c + 1], lambda c: sb_b2[:, c:c + 1],
                    lambda c: [x3[:, c, :]], pfx="ln2")
        for q2 in range(2):
            qs = slice(q2 * 256, (q2 + 1) * 256)
            layernorm_T(lambda c: x3[:, c, qs], 256,
                        lambda c: sb_gf[:, c:c + 1], lambda c: sb_bf[:, c:c + 1],
                        lambda c: [hfb[:, c, qs]], pfx=f"lnf{q2}")

        # ff1: m-groups of 4, c-outer inside the group so the chains start
        # as soon as hfb[c=0] lands (overlaps LNf normalize)
        relu = work.tile([P, 32, OWN], BF, tag="A32", name="relu")
        for m in range(32):
            w1_t = work.tile([P, C], BF, tag="W", bufs=5, name="w1_t")
            nc.sync.dma_start(out=w1_t[:], in_=d_w1[m])
            f_ps = ps.tile([P, OWN], F32, tag="bank", name="f_ps")
            for c in range(NT):
                nc.tensor.matmul(f_ps[:], w1_t[:, ts(c, P)], hfb[:, c, :],
                                 start=(c == 0), stop=(c == NT - 1))
            nc.scalar.activation(relu[:, m, :], f_ps[:], AF.Relu,
                                 bias=sb_b1f[:, m:m + 1])

        # ff2: single pass, 8 PSUM chains, w2 streamed once per m-tile
        o_ps = [ps.tile([P, OWN], F32, tag="bank", name=f"o_ps{j}")
                for j in range(8)]
        for m in range(32):
            w2t = work.tile([P, C], BF, tag="W", bufs=5, name="w2t")
            nc.sync.dma_start(out=w2t[:], in_=d_w2[m])
            for j in range(8):
                nc.tensor.matmul(o_ps[j][:], w2t[:, ts(j, P)], relu[:, m, :],
                                 start=(m == 0), stop=(m == 31))
        for co in range(8):
            t0 = work.tile([P, OWN], F32, tag="fft", bufs=2, name="fft")
            nc.vector.scalar_tensor_tensor(
                out=t0[:], in0=o_ps[co][:],
                scalar=sb_b2f[:, co:co + 1], in1=x3[:, co, :],
                op0=OP.add, op1=OP.add)
            out_sb = work.tile([P, OWN], F32, tag="osb", bufs=2, name="osb")
            nc.vector.tensor_add(out_sb[:], t0[:], hfb[:, co, :])
            nc.sync.dma_start(out=d_out[ts(co, P), :], in_=out_sb[:])

        ps.release()
        work.release()
        perst.release()
        const.release()

    nc.finalize()
    return nc


def _get_nc():
    if "nc" not in _CACHE:
        _CACHE["nc"] = _build_nc()
    return _CACHE["nc"]


def kernel(**inputs):
    from concourse.bass_utils import run_bass_kernel_spmd

    nc = _get_nc()
    bf16 = ml_dtypes.bfloat16

    f = {k: np.asarray(v, dtype=np.float32) for k, v in inputs.items()}
    x = f["x"]

    def stack_heads(w):  # [H, C, hd] -> [C, H*hd]
        return np.ascontiguousarray(w.transpose(1, 0, 2).reshape(C, C))

    def block_lhsT(w, nm):  # [C, nm*128] -> [nm, P, C] per-tile lhsT blocks
        return np.ascontiguousarray(
            w.reshape(NT, P, nm, P).transpose(2, 1, 0, 3).reshape(nm, P, NT * P))

    def part_scalar(v, n):  # [n*128] -> [128, n]
        return np.ascontiguousarray(v.reshape(-1).reshape(n, P).T)

    wq_full = stack_heads(f["Wq"])
    wk_full = stack_heads(f["Wk"])
    shared = {
        "wq": block_lhsT(wq_full, NT).astype(bf16),
        "wk": block_lhsT(wk_full, NT).astype(bf16),
        "wv": stack_heads(f["Wv"]).astype(bf16),
        "w1": block_lhsT(np.ascontiguousarray(f["W1"]), 32).astype(bf16),
        "w2": np.ascontiguousarray(f["W2"].reshape(32, P, C)).astype(bf16),
        "bq": part_scalar(f["bq"], NT),
        "bk": part_scalar(f["bk"], NT),
        "bvb": np.ascontiguousarray(np.broadcast_to(f["bv"].reshape(-1), (P, C))),
        "g1": part_scalar(f["g1"], NT),
        "b1": part_scalar(f["b1"], NT),
        "g2": part_scalar(f["g2"], NT),
        "b2": part_scalar(f["b2"], NT),
        "gf": part_scalar(f["gf"], NT),
        "bf": part_scalar(f["bf"], NT),
        "b1f": part_scalar(f["b1f"], 32),
        "b2f": part_scalar(f["b2f"], NT),
    }
    masks = {par: _mask01_for(par) for par in (0, 1)}
    perms = {par: _perm(par) for par in (0, 1)}

    in_maps = []
    for core in range(N_CORES):
        b, par = core // 2, core % 2
        xT = np.ascontiguousarray(x[b].T[:, perms[par]])
        in_maps.append({**shared, "xT": xT, "mask": masks[par]})

    res = run_bass_kernel_spmd(nc, in_maps, list(range(N_CORES)))

    out = np.empty((B, T, C), np.float32)
    for core in range(N_CORES):
        b, par = core // 2, core % 2
        outT = res.results[core]["outT"]  # [C, OWN]
        out[b, perms[par][:OWN], :] = outT.T
    return out



# revision 8
# speedup vs baseline: 1.1059x; 1.1059x over previous
"""Trainium2 Bass kernel: single transformer decoder block
(B=4, T=1024, C=1024, H=16 heads, head_dim=64, FFN hidden=4096).

Sharding: sequence-parallel across 8 NeuronCores = 4 batches x 2
causally-balanced token halves.  No collectives.

v2: fp8(e4m3) DoubleRow matmuls for QKV projections and both FFN
matmuls (weights pre-scaled x16 / x256 into fp8 range, rescale folded
into downstream activation scales); LN stats straight from fp32 input
via f32r-bitcast matmuls (no bf16 pre-copy); sum/sq stat chains on
separate PE column groups; attention emission software-pipelined
(scores of head-pair hp+1 issued before AV of hp) so the PE never
idles past the HAM window.

Layout notes: everything transposed on-chip [C on partitions, tokens
on free dim]; scores computed transposed (S^T = K^T-stationary x Q^T);
causal masking by multiplicative bf16 0/1 mask on exp; softmax
denominator via 16.0-valued ones-matmul (cancels the x16 of V).
"""

import numpy as np
import ml_dtypes

B, T, C, H = 4, 1024, 1024, 16
HD = 64
FF = 4096
P = 128
NT = 8  # token tiles == C tiles
OWN = 512
N_CORES = 8
ATT_SCALE = 1.0 / (32.0 * 256.0)  # 1/sqrt(C) / (16*16 qk weight scale)

# own q-tiles per parity, sorted by descending causal span
OWN_TILES = {0: [7, 4, 3, 0], 1: [6, 5, 2, 1]}
# active slot count per s-position p (p<4: own tiles, p>=4: partner tiles)
NP_ACT = [1, 2, 3, 4, 1, 2, 3, 4]

_CACHE = {}


def _perm(par):
    tiles = OWN_TILES[par] + OWN_TILES[1 - par]
    return np.concatenate([np.arange(k * P, (k + 1) * P) for k in tiles])


def _mask01_for(par):
    m = np.ones((P, NT, P), np.float32)
    tri = (np.arange(P)[:, None] <= np.arange(P)[None, :]).astype(np.float32)
    for p in range(4):
        m[:, p, :] = tri
    for j in range(4):
        if OWN_TILES[1 - par][j] > OWN_TILES[par][j]:
            m[:, 4 + j, :] = 0.0
    return m.astype(ml_dtypes.bfloat16)


def _build_nc():
    import concourse.bass as bass
    from concourse import bacc
    import concourse.mybir as mybir
    import concourse.tile as tile

    AF = mybir.ActivationFunctionType
    OP = mybir.AluOpType
    F32 = mybir.dt.float32
    F32R = mybir.dt.float32r
    BF = mybir.dt.bfloat16
    F8 = mybir.dt.float8e4
    DR = mybir.MatmulPerfMode.DoubleRow
    ts = bass.ts

    nc = bacc.Bacc()
    d_xT = nc.dram_tensor("xT", [C, T], F32R, kind="ExternalInput")
    # blocked weights: see kernel() for the host-side layouts
    d_wq = nc.dram_tensor("wq", [NT, P, NT, P], F8, kind="ExternalInput")
    d_wk = nc.dram_tensor("wk", [NT, P, NT, P], F8, kind="ExternalInput")
    d_wv = nc.dram_tensor("wv", [C, C], F8, kind="ExternalInput")
    d_w1 = nc.dram_tensor("w1", [32, P, NT, P], F8, kind="ExternalInput")
    d_w2 = nc.dram_tensor("w2", [16, P, 2, C], F8, kind="ExternalInput")
    d_bq = nc.dram_tensor("bq", [P, NT], F32, kind="ExternalInput")
    d_bk = nc.dram_tensor("bk", [P, NT], F32, kind="ExternalInput")
    d_bvb = nc.dram_tensor("bvb", [P, C], F32, kind="ExternalInput")
    d_g1 = nc.dram_tensor("g1", [P, NT], F32, kind="ExternalInput")
    d_b1 = nc.dram_tensor("b1", [P, NT], F32, kind="ExternalInput")
    d_g2 = nc.dram_tensor("g2", [P, NT], F32, kind="ExternalInput")
    d_b2 = nc.dram_tensor("b2", [P, NT], F32, kind="ExternalInput")
    d_gf = nc.dram_tensor("gf", [P, NT], F32, kind="ExternalInput")
    d_bf = nc.dram_tensor("bf", [P, NT], F32, kind="ExternalInput")
    d_b1f = nc.dram_tensor("b1f", [P, 32], F32, kind="ExternalInput")
    d_b2f = nc.dram_tensor("b2f", [P, NT], F32, kind="ExternalInput")
    d_mask = nc.dram_tensor("mask", [P, NT, P], BF, kind="ExternalInput")
    d_ones = nc.dram_tensor("onesr", [P, 1], F32R, kind="ExternalInput")
    d_out = nc.dram_tensor("outT", [C, OWN], F32, kind="ExternalOutput")

    with tile.TileContext(nc) as tc:
        const = tc.alloc_tile_pool(name="const", bufs=1)
        perst = tc.alloc_tile_pool(name="perst", bufs=1)
        work = tc.alloc_tile_pool(name="work", bufs=1)
        ps = tc.alloc_tile_pool(name="ps", bufs=8, space="PSUM")

        ones_bf = const.tile([P, P], BF)
        nc.vector.memset(ones_bf[:], 1.0)
        s16_bf = const.tile([P, 64], BF)
        nc.vector.memset(s16_bf[:], 16.0)
        ones_r = const.tile([P, 1], F32R)
        nc.sync.dma_start(out=ones_r[:], in_=d_ones[:])
        eps1 = const.tile([1, 1], F32)
        nc.vector.memset(eps1[:], 1e-5)

        # HAM warm-up: ~3.5us of dummy matmuls while input DMAs land
        warm = ps.tile([1, P], F32, tag="bank", name="warm")
        for _ in range(32):
            nc.tensor.matmul(warm[:], ones_bf[:, 0:1], ones_bf[:, :],
                             start=True, stop=True)

        def load_const(name, dram, shape, dt=F32):
            t = const.tile(shape, dt, name=name)
            nc.sync.dma_start(out=t[:], in_=dram[:])
            return t

        xt = work.tile([P, NT, T], F32R, tag="A32", name="xt")
        sb_wv = work.tile([P, NT, C], F8, tag="B16", name="sb_wv")
        for h2 in range(2):
            for c in range(NT):
                nc.sync.dma_start(out=xt[:, c, ts(h2, 512)],
                                  in_=d_xT[ts(c, P), ts(h2, 512)])
        for c in range(NT):
            nc.sync.dma_start(out=sb_wv[:, c, :], in_=d_wv[ts(c, P), :])

        sb_bq = load_const("sb_bq", d_bq, [P, NT])
        sb_bk = load_const("sb_bk", d_bk, [P, NT])
        sb_bvb = load_const("sb_bvb", d_bvb, [P, C])
        sb_g1 = load_const("sb_g1", d_g1, [P, NT])
        sb_b1 = load_const("sb_b1", d_b1, [P, NT])
        sb_g2 = load_const("sb_g2", d_g2, [P, NT])
        sb_b2 = load_const("sb_b2", d_b2, [P, NT])
        sb_gf = load_const("sb_gf", d_gf, [P, NT])
        sb_bf = load_const("sb_bf", d_bf, [P, NT])
        sb_b1f = load_const("sb_b1f", d_b1f, [P, 32])
        sb_b2f = load_const("sb_b2f", d_b2f, [P, NT])
        sb_mask = load_const("sb_mask", d_mask, [P, NT, P], BF)

        # persistent activations
        hbf = perst.tile([P, NT, T], BF)
        h8 = perst.tile([P, NT, T], F8)
        kT = perst.tile([P, NT, T], BF)
        vT = perst.tile([P, NT, C], BF)
        qT = perst.tile([P, NT, OWN], BF)
        x3 = perst.tile([P, NT, OWN], F32R)

        def ln_stat(src_ap, cols, sum_ps, sq_ps, first, last, pfx):
            # sum chain from fp32 source via f32r bitcast; sq via bf16
            sq = work.tile([P, 512], BF, tag="sq", bufs=2, name=f"{pfx}sqt")[:, 0:cols]
            nc.vector.tensor_mul(sq[:], src_ap, src_ap)
            nc.tensor.matmul(sum_ps, ones_r[:, 0:1],
                             src_ap, start=first, stop=last)
            nc.tensor.matmul(sq_ps, ones_bf[:, 0:1], sq[:], start=first, stop=last)

        def ln_finish(sum_ps, sq_ps, src, cols, g_ap, b_ap, outs, pfx):
            mu = work.tile([1, 512], F32, tag="mu", bufs=1, name=f"{pfx}mu")[:, 0:cols]
            t2 = work.tile([1, 512], F32, tag="t2", bufs=1, name=f"{pfx}t2")[:, 0:cols]
            rstd = work.tile([1, 512], F32, tag="rstd", bufs=1, name=f"{pfx}rstd")[:, 0:cols]
            nc.scalar.activation(mu[:], sum_ps, AF.Copy, scale=1.0 / C)
            nc.scalar.activation(t2[:], sq_ps, AF.Copy, scale=1.0 / C)
            nc.vector.tensor_mul(rstd[:], mu[:], mu[:])
            nc.vector.tensor_sub(t2[:], t2[:], rstd[:])
            nc.scalar.activation(t2[:], t2[:], AF.Sqrt, bias=eps1[:])
            nc.vector.reciprocal_approx_fast(out=rstd[:], in_=t2[:])
            mu_bf = work.tile([1, 512], BF, tag="mubf", bufs=2, name=f"{pfx}mubf")[:, 0:cols]
            rs_bf = work.tile([1, 512], BF, tag="rsbf", bufs=2, name=f"{pfx}rsbf")[:, 0:cols]
            nc.scalar.activation(mu_bf[:], mu[:], AF.Copy)
            nc.scalar.activation(rs_bf[:], rstd[:], AF.Copy)
            mu_bc = ps.tile([P, cols], F32, tag="bank", name=f"{pfx}mubc")
            rs_bc = ps.tile([P, cols], F32, tag="bank", name=f"{pfx}rsbc")
            nc.tensor.matmul(mu_bc[:], ones_bf[0:1, 0:P], mu_bf[0:1, :],
                             start=True, stop=True)
            nc.tensor.matmul(rs_bc[:], ones_bf[0:1, 0:P], rs_bf[0:1, :],
                             start=True, stop=True)
            for c in range(NT):
                s = src(c)
                t0 = work.tile([P, 512], F32, tag="t0", bufs=2, name=f"{pfx}t0")[:, 0:cols]
                nc.vector.tensor_sub(t0[:], s, mu_bc[:])
                nc.vector.tensor_mul(t0[:], t0[:], rs_bc[:])
                for dst in outs(c):
                    nc.vector.tensor_scalar(
                        out=dst, in0=t0[:],
                        scalar1=g_ap(c), scalar2=b_ap(c),
                        op0=OP.mult, op1=OP.add)

        def layernorm_T(src, cols, g_ap, b_ap, outs, pfx):
            # sum chain at PE column group 0, sq chain at column group 64
            sum_t = ps.tile([P, cols], F32, tag="bank", name=f"{pfx}sum")
            sq_t = ps.tile([P, cols], F32, tag="bank", name=f"{pfx}sq")
            sum_ps = sum_t[0:1, :]
            sq_ps = sq_t[64:65, :]
            for c in range(NT):
                ln_stat(src(c), cols, sum_ps, sq_ps, c == 0, c == NT - 1, pfx)
            ln_finish(sum_ps, sq_ps, src, cols, g_ap, b_ap, outs, pfx)

        # ---------------- Phase 1: LN1 (split halves) + QKV projections
        for h2 in range(2):
            cs = slice(h2 * 512, (h2 + 1) * 512)
            layernorm_T(lambda c: xt[:, c, cs], 512,
                        lambda c: sb_g1[:, c:c + 1], lambda c: sb_b1[:, c:c + 1],
                        lambda c: [hbf[:, c, cs]], pfx=f"ln1{h2}")
            for c in range(NT):
                nc.scalar.activation(h8[:, c, cs], hbf[:, c, cs], AF.Copy)
            # v token-major for this half's token tiles (fp8 DoubleRow)
            for t in range(h2 * 4, h2 * 4 + 4):
                v_ps0 = ps.tile([P, 512], F32, tag="bank", name="v_ps0")
                v_ps1 = ps.tile([P, 512], F32, tag="bank", name="v_ps1")
                for cp in range(0, NT, 2):
                    first, last = cp == 0, cp == NT - 2
                    lhsT = h8[:, cp:cp + 2, ts(t, P)]
                    nc.tensor.matmul(v_ps0[:], lhsT, sb_wv[:, cp:cp + 2, 0:512],
                                     start=first, stop=last, perf_mode=DR)
                    nc.tensor.matmul(v_ps1[:], lhsT, sb_wv[:, cp:cp + 2, 512:1024],
                                     start=first, stop=last, perf_mode=DR)
                nc.vector.tensor_add(vT[:, t, 0:512], v_ps0[:], sb_bvb[:, 0:512])
                nc.vector.tensor_add(vT[:, t, 512:1024], v_ps1[:], sb_bvb[:, 512:1024])
            if h2 == 0:
                # qT[d, own tokens] (fp8 DoubleRow)
                for hp in range(NT):
                    wq_t = work.tile([P, NT, P], F8, tag="W", bufs=5, name="wq_t")
                    nc.sync.dma_start(out=wq_t[:], in_=d_wq[hp])
                    q_ps = ps.tile([P, OWN], F32, tag="bank", name="q_ps")
                    for cp in range(0, NT, 2):
                        nc.tensor.matmul(q_ps[:], wq_t[:, cp:cp + 2, :],
                                         h8[:, cp:cp + 2, 0:OWN],
                                         start=(cp == 0), stop=(cp == NT - 2),
                                         perf_mode=DR)
                    nc.vector.tensor_scalar(out=qT[:, hp, :], in0=q_ps[:],
                                            scalar1=sb_bq[:, hp:hp + 1], scalar2=None,
                                            op0=OP.add)
        # kT[d, all tokens] (fp8 DoubleRow, overlaps attention start)
        for hp in range(NT):
            wk_t = work.tile([P, NT, P], F8, tag="W", bufs=5, name="wk_t")
            nc.sync.dma_start(out=wk_t[:], in_=d_wk[hp])
            k_ps0 = ps.tile([P, 512], F32, tag="bank", name="k_ps0")
            k_ps1 = ps.tile([P, 512], F32, tag="bank", name="k_ps1")
            for cp in range(0, NT, 2):
                first, last = cp == 0, cp == NT - 2
                lhsT = wk_t[:, cp:cp + 2, :]
                nc.tensor.matmul(k_ps0[:], lhsT, h8[:, cp:cp + 2, 0:512],
                                 start=first, stop=last, perf_mode=DR)
                nc.tensor.matmul(k_ps1[:], lhsT, h8[:, cp:cp + 2, 512:1024],
                                 start=first, stop=last, perf_mode=DR)
            nc.vector.tensor_scalar(out=kT[:, hp, 0:512], in0=k_ps0[:],
                                    scalar1=sb_bk[:, hp:hp + 1], scalar2=None,
                                    op0=OP.add)
            nc.vector.tensor_scalar(out=kT[:, hp, 512:1024], in0=k_ps1[:],
                                    scalar1=sb_bk[:, hp:hp + 1], scalar2=None,
                                    op0=OP.add)

        # ---------------- Phase 2: attention (software-pipelined) --------
        x2 = work.tile([P, NT, OWN], F32R, tag="A32", name="x2")
        CUM = [0, 128, 384, 768, 1280, 1408, 1664, 2048]  # packed exp offsets

        def emit_scores(hp):
            ex_e = work.tile([P, 2560], BF, tag="exe", bufs=2, name="ex_e")
            ex_o = work.tile([P, 2560], BF, tag="exo", bufs=2, name="ex_o")
            for p in range(NT):
                w = NP_ACT[p] * P
                po = CUM[p]
                st_e = ps.tile([P, OWN], F32, tag="bank", name="st_e")
                st_o = ps.tile([P, OWN], F32, tag="bank", name="st_o")
                nc.tensor.matmul(st_e[:, 0:w], kT[0:64, hp, ts(p, P)],
                                 qT[0:64, hp, 0:w], start=True, stop=True)
                nc.tensor.matmul(st_o[:, 0:w], kT[64:128, hp, ts(p, P)],
                                 qT[64:128, hp, 0:w], start=True, stop=True)
                nc.scalar.activation(ex_e[:, po:po + w], st_e[:, 0:w], AF.Exp,
                                     scale=ATT_SCALE)
                nc.scalar.activation(ex_o[:, po:po + w], st_o[:, 0:w], AF.Exp,
                                     scale=ATT_SCALE)
                im = p % 4
                mw = slice(po + im * P, po + (im + 1) * P)
                nc.vector.tensor_mul(ex_e[:, mw], ex_e[:, mw], sb_mask[:, p, :])
                nc.vector.tensor_mul(ex_o[:, mw], ex_o[:, mw], sb_mask[:, p, :])
            return ex_e, ex_o

        def emit_av(hp, ex_e, ex_o):
            gat = work.tile([P, OWN], F32, tag="gat", bufs=1, name="gat")
            rs = work.tile([P, OWN], F32, tag="rs", bufs=1, name="rs")
            oTe = ps.tile([64, OWN], F32, tag="bank", name="oTe")
            oTo = ps.tile([P, OWN], F32, tag="bank", name="oTo")
            seE = ps.tile([64, OWN], F32, tag="bank", name="seE")
            seO = ps.tile([P, OWN], F32, tag="bank", name="seO")
            for p in [3, 0, 1, 2, 4, 5, 6, 7]:
                w = NP_ACT[p] * P
                first = (p == 3)
                last = (p == NT - 1)
                pc = slice(CUM[p], CUM[p] + w)
                nc.tensor.matmul(oTe[:, 0:w], vT[:, p, hp * P:hp * P + 64],
                                 ex_e[:, pc], start=first, stop=last)
                nc.tensor.matmul(oTo[64:128, 0:w],
                                 vT[:, p, hp * P + 64:hp * P + 128],
                                 ex_o[:, pc], start=first, stop=last)
                nc.tensor.matmul(seE[:, 0:w], s16_bf[:, :],
                                 ex_e[:, pc], start=first, stop=last)
                nc.tensor.matmul(seO[64:128, 0:w], s16_bf[:, :],
                                 ex_o[:, pc], start=first, stop=last)
            nc.vector.tensor_copy(gat[0:64, :], seE[:, :])
            nc.vector.tensor_copy(gat[64:128, :], seO[64:128, :])
            nc.vector.reciprocal_approx_fast(out=rs[:], in_=gat[:])
            ot = work.tile([P, OWN], F32, tag="ot", bufs=1, name="ot")
            nc.vector.tensor_mul(ot[0:64, :], oTe[:, :], rs[0:64, :])
            nc.vector.tensor_mul(ot[64:128, :], oTo[64:128, :], rs[64:128, :])
            # x2 = h + attn_out (residual uses post-LN h); oT/den 16s cancel
            nc.vector.tensor_add(x2[:, hp, :], ot[:, :], hbf[:, hp, 0:OWN])

        ex_cur = emit_scores(0)
        for hp in range(NT):
            ex_nxt = emit_scores(hp + 1) if hp + 1 < NT else None
            emit_av(hp, *ex_cur)
            ex_cur = ex_nxt

        # ---------------- Phase 3: LN2, LNf, FFN ----------------
        hfb = work.tile([P, NT, OWN], BF, tag="C16", name="hfb")
        hf8 = work.tile([P, NT, OWN], F8, tag="D8", name="hf8")
        layernorm_T(lambda c: x2[:, c, :], 512,
                    lambda c: sb_g2[:, c:c + 1], lambda c: sb_b2[:, c:c + 1],
                    lambda c: [x3[:, c, :]], pfx="ln2")
        for q2 in range(2):
            qs = slice(q2 * 256, (q2 + 1) * 256)
            layernorm_T(lambda c: x3[:, c, qs], 256,
                        lambda c: sb_gf[:, c:c + 1], lambda c: sb_bf[:, c:c + 1],
                        lambda c: [hfb[:, c, qs]], pfx=f"lnf{q2}")
            for c in range(NT):
                nc.scalar.activation(hf8[:, c, qs], hfb[:, c, qs], AF.Copy)

        # ff1 (fp8 DoubleRow), relu output straight to fp8 with 1/256 fold
        relu8 = work.tile([P, 32, OWN], F8, tag="A32", name="relu8")
        for m in range(32):
            w1_t = work.tile([P, NT, P], F8, tag="W", bufs=5, name="w1_t")
            nc.sync.dma_start(out=w1_t[:], in_=d_w1[m])
            f_ps = ps.tile([P, OWN], F32, tag="bank", name="f_ps")
            for cp in range(0, NT, 2):
                nc.tensor.matmul(f_ps[:], w1_t[:, cp:cp + 2, :],
                                 hf8[:, cp:cp + 2, :],
                                 start=(cp == 0), stop=(cp == NT - 2),
                                 perf_mode=DR)
            nc.scalar.activation(relu8[:, m, :], f_ps[:], AF.Relu,
                                 bias=sb_b1f[:, m:m + 1], scale=1.0 / 256.0)

        # ff2 (fp8 DoubleRow over m-pairs), 8 PSUM chains
        o_ps = [ps.tile([P, OWN], F32, tag="bank", name=f"o_ps{j}")
                for j in range(8)]
        for mb in range(16):
            w2t = work.tile([P, 2, C], F8, tag="W", bufs=5, name="w2t")
            nc.sync.dma_start(out=w2t[:], in_=d_w2[mb])
            for j in range(8):
                nc.tensor.matmul(o_ps[j][:], w2t[:, :, ts(j, P)],
                                 relu8[:, 2 * mb:2 * mb + 2, :],
                                 start=(mb == 0), stop=(mb == 15),
                                 perf_mode=DR)
        for co in range(8):
            u = work.tile([P, OWN], F32, tag="fft", bufs=2, name="fft")
            nc.scalar.activation(u[:], o_ps[co][:], AF.Identity,
                                 bias=sb_b2f[:, co:co + 1], scale=1.0 / 256.0)
            nc.vector.tensor_add(u[:], u[:], x3[:, co, :])
            out_sb = work.tile([P, OWN], F32, tag="osb", bufs=2, name="osb")
            nc.vector.tensor_add(out_sb[:], u[:], hfb[:, co, :])
            nc.sync.dma_start(out=d_out[ts(co, P), :], in_=out_sb[:])

        ps.release()
        work.release()
        perst.release()
        const.release()

    nc.finalize()
    return nc


def _get_nc():
    if "nc" not in _CACHE:
        _CACHE["nc"] = _build_nc()
    return _CACHE["nc"]


def kernel(**inputs):
    from concourse.bass_utils import run_bass_kernel_spmd

    nc = _get_nc()
    bf16 = ml_dtypes.bfloat16
    e4 = ml_dtypes.float8_e4m3

    def q8(x):
        return np.clip(np.asarray(x, np.float32), -240.0, 240.0).astype(e4)

    f = {k: np.asarray(v, dtype=np.float32) for k, v in inputs.items()}
    x = f["x"]

    def stack_heads(w):  # [H, C, hd] -> [C, H*hd]
        return np.ascontiguousarray(w.transpose(1, 0, 2).reshape(C, C))

    def block_lhsT(w, nm):  # [C, nm*128] -> [nm, P, NT, P] per-tile lhsT blocks
        return np.ascontiguousarray(
            w.reshape(NT, P, nm, P).transpose(2, 1, 0, 3))

    def part_scalar(v, n):  # [n*128] -> [128, n]
        return np.ascontiguousarray(v.reshape(-1).reshape(n, P).T)

    wq_full = stack_heads(f["Wq"]) * 16.0
    wk_full = stack_heads(f["Wk"]) * 16.0
    w2_pair = np.ascontiguousarray(
        (f["W2"] * 256.0).reshape(16, 2, P, C).transpose(0, 2, 1, 3))
    shared = {
        "wq": q8(block_lhsT(wq_full, NT)),
        "wk": q8(block_lhsT(wk_full, NT)),
        "wv": q8(stack_heads(f["Wv"]) * 16.0),
        "w1": q8(block_lhsT(np.ascontiguousarray(f["W1"]) * 256.0, 32)),
        "w2": q8(w2_pair),
        "bq": part_scalar(f["bq"] * 16.0, NT),
        "bk": part_scalar(f["bk"] * 16.0, NT),
        "bvb": np.ascontiguousarray(
            np.broadcast_to(f["bv"].reshape(-1) * 16.0, (P, C))),
        "g1": part_scalar(f["g1"], NT),
        "b1": part_scalar(f["b1"], NT),
        "g2": part_scalar(f["g2"], NT),
        "b2": part_scalar(f["b2"], NT),
        "gf": part_scalar(f["gf"], NT),
        "bf": part_scalar(f["bf"], NT),
        "b1f": part_scalar(f["b1f"], 32),
        "b2f": part_scalar(f["b2f"], NT),
    }
    shared["onesr"] = np.ones((P, 1), np.float32)
    masks = {par: _mask01_for(par) for par in (0, 1)}
    perms = {par: _perm(par) for par in (0, 1)}

    in_maps = []
    for core in range(N_CORES):
        b, par = core // 2, core % 2
        xT = np.ascontiguousarray(x[b].T[:, perms[par]])
        in_maps.append({**shared, "xT": xT, "mask": masks[par]})

    res = run_bass_kernel_spmd(nc, in_maps, list(range(N_CORES)))

    out = np.empty((B, T, C), np.float32)
    for core in range(N_CORES):
        b, par = core // 2, core % 2
        outT = res.results[core]["outT"]  # [C, OWN]
        out[b, perms[par][:OWN], :] = outT.T
    return out


# revision 25
# speedup vs baseline: 1.3280x; 1.2009x over previous
"""Trainium2 Bass kernel: single transformer decoder block
(B=4, T=1024, C=1024, H=16 heads, head_dim=64, FFN hidden=4096).

Sharding: sequence-parallel across 8 NeuronCores = 4 batches x 2
causally-balanced token halves.  No collectives.

v3a on top of v2 (fp8 DoubleRow QKV/FFN, f32r LN stats, pipelined
attention emission):
  - LN2+LNf fused: both layernorms' statistics come from one pass over
    x2 (weighted stat matmuls with lhsT [1,g2,g2^2,g2*b2]), second-LN
    moments derived algebraically; applies are 2-op (mul + stt) with
    outer-product bias broadcasts.
  - LN1 apply also 2-op via the same broadcast trick.
  - FF2 split into two output-column groups so evacuation/DMA of group
    0 overlaps group 1's matmuls.
  - Dummy-matmul bridge packs across LN serial chains keep the PE HAM
    clock at 8/8; K-projection over the h2=0 half is emitted as the
    natural bridge across LN1-h1's finish chain.
"""

import numpy as np
import ml_dtypes

B, T, C, H = 4, 1024, 1024, 16
HD = 64
FF = 4096
P = 128
NT = 8  # token tiles == C tiles
OWN = 512
N_CORES = 8
ATT_SCALE = 1.0 / (32.0 * 256.0)  # 1/sqrt(C) / (16*16 qk weight scale)

OWN_TILES = {0: [7, 4, 3, 0], 1: [6, 5, 2, 1]}
NP_ACT = [1, 2, 3, 4, 1, 2, 3, 4]

_CACHE = {}


def _perm(par):
    tiles = OWN_TILES[par] + OWN_TILES[1 - par]
    return np.concatenate([np.arange(k * P, (k + 1) * P) for k in tiles])


def _mask01_for(par):
    m = np.ones((P, NT, P), np.float32)
    tri = (np.arange(P)[:, None] <= np.arange(P)[None, :]).astype(np.float32)
    for p in range(4):
        m[:, p, :] = tri
    for j in range(4):
        if OWN_TILES[1 - par][j] > OWN_TILES[par][j]:
            m[:, 4 + j, :] = 0.0
    return m.astype(ml_dtypes.bfloat16)


def _build_nc():
    import concourse.bass as bass
    from concourse import bacc
    import concourse.mybir as mybir
    import concourse.tile as tile

    AF = mybir.ActivationFunctionType
    OP = mybir.AluOpType
    F32 = mybir.dt.float32
    F32R = mybir.dt.float32r
    BF = mybir.dt.bfloat16
    F8 = mybir.dt.float8e4
    DR = mybir.MatmulPerfMode.DoubleRow
    ts = bass.ts

    nc = bacc.Bacc()
    d_xT = nc.dram_tensor("xT", [C, T], F32R, kind="ExternalInput")
    d_wq = nc.dram_tensor("wq", [NT, P, NT, P], F8, kind="ExternalInput")
    d_wk = nc.dram_tensor("wk", [NT, P, NT, P], F8, kind="ExternalInput")
    d_wv = nc.dram_tensor("wv", [C, C], F8, kind="ExternalInput")
    d_w1 = nc.dram_tensor("w1", [32, P, NT, P], F8, kind="ExternalInput")
    d_w2 = nc.dram_tensor("w2", [2, 16, P, 2, 512], F8, kind="ExternalInput")
    d_bq = nc.dram_tensor("bq", [P, NT], F32, kind="ExternalInput")
    d_bk = nc.dram_tensor("bk", [P, NT], F32, kind="ExternalInput")
    d_bvb = nc.dram_tensor("bvb", [P, C], F32, kind="ExternalInput")
    d_b1f = nc.dram_tensor("b1f", [P, 32], F32, kind="ExternalInput")
    d_b2f = nc.dram_tensor("b2f", [P, NT], F32, kind="ExternalInput")
    d_g1 = nc.dram_tensor("g1", [P, NT], F32, kind="ExternalInput")
    # fused-LN tables: per-column weighted-stat lhsT vectors,
    # gbt[:,L] = [g_L; b_L] K=2 lhsT rows, sc = host scalar moments
    d_statg = nc.dram_tensor("statg", [P, NT], F32R, kind="ExternalInput")
    d_statg2 = nc.dram_tensor("statg2", [P, NT], F32R, kind="ExternalInput")
    d_statgb = nc.dram_tensor("statgb", [P, NT], F32R, kind="ExternalInput")
    d_statq2 = nc.dram_tensor("statq2", [P, NT], BF, kind="ExternalInput")
    d_gbt = nc.dram_tensor("gbt", [2, 3, NT, P], BF, kind="ExternalInput")
    d_sc = nc.dram_tensor("sc", [1, 8], F32, kind="ExternalInput")
    d_g2 = nc.dram_tensor("g2", [P, NT], F32, kind="ExternalInput")
    d_gf = nc.dram_tensor("gf", [P, NT], F32, kind="ExternalInput")
    d_mask = nc.dram_tensor("mask", [P, NT, P], BF, kind="ExternalInput")
    d_ones = nc.dram_tensor("onesr", [P, 1], F32R, kind="ExternalInput")
    d_out = nc.dram_tensor("outT", [C, OWN], F32, kind="ExternalOutput")

    with tile.TileContext(nc) as tc:
        const = tc.alloc_tile_pool(name="const", bufs=1)
        perst = tc.alloc_tile_pool(name="perst", bufs=1)
        work = tc.alloc_tile_pool(name="work", bufs=1)
        ps = tc.alloc_tile_pool(name="ps", bufs=8, space="PSUM")

        ones_bf = const.tile([P, P], BF)
        nc.vector.memset(ones_bf[:], 1.0)
        s16_bf = const.tile([P, 64], BF)
        nc.vector.memset(s16_bf[:], 16.0)
        ones_r = const.tile([P, 1], F32R)
        nc.sync.dma_start(out=ones_r[:], in_=d_ones[:])
        eps1 = const.tile([1, 1], F32)
        nc.vector.memset(eps1[:], 1e-5)

        def warm_pack(n, name="warm"):
            wt = ps.tile([1, P], F32, tag="bank", name=name)
            for _ in range(n):
                nc.tensor.matmul(wt[:], ones_bf[:, 0:1], ones_bf[:, :],
                                 start=True, stop=True)

        warm_pack(32)

        def load_const(name, dram, shape, dt=F32):
            t = const.tile(shape, dt, name=name)
            nc.sync.dma_start(out=t[:], in_=dram[:])
            return t

        xt = work.tile([P, NT, T], F32R, tag="A32", name="xt")
        sb_wv = work.tile([P, NT, C], F8, tag="B16", name="sb_wv")
        for h2 in range(2):
            for c in range(NT):
                nc.sync.dma_start(out=xt[:, c, ts(h2, 512)],
                                  in_=d_xT[ts(c, P), ts(h2, 512)])
        for c in range(NT):
            nc.sync.dma_start(out=sb_wv[:, c, :], in_=d_wv[ts(c, P), :])

        sb_bq = load_const("sb_bq", d_bq, [P, NT])
        sb_bk = load_const("sb_bk", d_bk, [P, NT])
        sb_bvb = load_const("sb_bvb", d_bvb, [P, C])
        sb_b1f = load_const("sb_b1f", d_b1f, [P, 32])
        sb_b2f = load_const("sb_b2f", d_b2f, [P, NT])
        sb_g1 = load_const("sb_g1", d_g1, [P, NT])
        sb_g2 = load_const("sb_g2", d_g2, [P, NT])
        sb_gf = load_const("sb_gf", d_gf, [P, NT])
        sb_statg = load_const("sb_statg", d_statg, [P, NT], F32R)
        sb_statg2 = load_const("sb_statg2", d_statg2, [P, NT], F32R)
        sb_statgb = load_const("sb_statgb", d_statgb, [P, NT], F32R)
        sb_statq2 = load_const("sb_statq2", d_statq2, [P, NT], BF)
        sb_gbt = load_const("sb_gbt", d_gbt, [2, 3, NT, P], BF)
        sb_sc = load_const("sb_sc", d_sc, [1, 8])
        sb_mask = load_const("sb_mask", d_mask, [P, NT, P], BF)

        # persistent activations
        hbf = perst.tile([P, NT, T], BF)
        h8 = perst.tile([P, NT, T], F8)
        kT = perst.tile([P, NT, T], BF)
        vT = perst.tile([P, NT, C], BF)
        qT = perst.tile([P, NT, OWN], BF)
        x3 = perst.tile([P, NT, OWN], F32R)

        def small(pfx, name, dt=F32, bufs=1):
            return work.tile([1, 512], dt, tag=name, bufs=bufs,
                             name=f"{pfx}{name}")

        def ln_core(sum_ps, sq_ps, cols, pfx):
            """PSUM row sums -> (mu[1,cols] f32, r[1,cols] f32 tiles)."""
            mu = small(pfx, "mu")[:, 0:cols]
            t2 = small(pfx, "t2")[:, 0:cols]
            r = small(pfx, "rstd")[:, 0:cols]
            nc.scalar.activation(mu[:], sum_ps, AF.Copy, scale=1.0 / C)
            nc.scalar.activation(t2[:], sq_ps, AF.Copy, scale=1.0 / C)
            nc.vector.tensor_mul(r[:], mu[:], mu[:])
            nc.vector.tensor_sub(t2[:], t2[:], r[:])
            nc.scalar.activation(t2[:], t2[:], AF.Sqrt, bias=eps1[:])
            nc.vector.reciprocal_approx_fast(out=r[:], in_=t2[:])
            return mu, r

        def bc_setup(mu, r, cols, pfx, tagix=0):
            """Broadcast r over partitions (A_bc psum) and build the mB
            rhs [mu*r; -1] for the per-tile outer-product bias matmuls."""
            rs_bf = small(pfx, "rsbf", BF, bufs=1)[:, 0:cols]
            nc.scalar.activation(rs_bf[:], r[:], AF.Copy)
            mB = work.tile([2, 512], BF, tag=f"mB{tagix}", bufs=1,
                           name=f"{pfx}mB")[:, 0:cols]
            nc.vector.memset(mB[:, :], -1.0)
            nc.vector.tensor_mul(mB[0:1, :], mu[:], r[:])
            A_bc = ps.tile([P, cols], F32, tag="bank", name=f"{pfx}Abc")
            nc.tensor.matmul(A_bc[:], ones_bf[0:1, 0:P], rs_bf[0:1, :],
                             start=True, stop=True)
            return A_bc, mB

        def ln_apply(c, src_ap, dst, A_bc, mB, L, g_col, cols, pfx):
            """dst = (src*A)*g - (g*(mu*r) - b), per c-tile."""
            B_ps = ps.tile([P, cols], F32, tag="bank", name=f"{pfx}Bps")
            nc.tensor.matmul(B_ps[:], sb_gbt[:, L, c, :], mB[:, :],
                             start=True, stop=True)
            t1 = work.tile([P, 512], F32, tag="t0", bufs=2,
                           name=f"{pfx}t1")[:, 0:cols]
            nc.vector.tensor_mul(t1[:], src_ap, A_bc[:])
            nc.vector.scalar_tensor_tensor(
                out=dst, in0=t1[:], scalar=g_col,
                op0=OP.mult, in1=B_ps[:], op1=OP.subtract)

        # ---------------- Phase 1: LN1 + QKV projections ----------------
        def ln1_stats(h2):
            cs = slice(h2 * 512, (h2 + 1) * 512)
            sum_t = ps.tile([P, 512], F32, tag="bank", name=f"l1s{h2}")
            sq_t = ps.tile([P, 512], F32, tag="bank", name=f"l1q{h2}")
            for c in range(NT):
                sq = work.tile([P, 512], BF, tag="sq", bufs=2, name="sq")
                nc.vector.tensor_mul(sq[:], xt[:, c, cs], xt[:, c, cs])
                nc.tensor.matmul(sum_t[0:1, :], ones_r[:, 0:1], xt[:, c, cs],
                                 start=(c == 0), stop=(c == NT - 1))
                nc.tensor.matmul(sq_t[64:65, :], ones_bf[:, 0:1], sq[:],
                                 start=(c == 0), stop=(c == NT - 1))
            return sum_t[0:1, :], sq_t[64:65, :]

        def ln1_apply(h2, sum_ps, sq_ps):
            cs = slice(h2 * 512, (h2 + 1) * 512)
            mu, r = ln_core(sum_ps, sq_ps, 512, f"ln1{h2}")
            A_bc, mB = bc_setup(mu, r, 512, f"ln1{h2}")
            for c in range(NT):
                ln_apply(c, xt[:, c, cs], hbf[:, c, cs], A_bc, mB, 0,
                         sb_g1[:, c:c + 1], 512, f"ln1{h2}")
                nc.scalar.activation(h8[:, c, cs], hbf[:, c, cs], AF.Copy)

        def v_proj(t):
            v_ps0 = ps.tile([P, 512], F32, tag="bank", name="v_ps0")
            v_ps1 = ps.tile([P, 512], F32, tag="bank", name="v_ps1")
            for cp in range(0, NT, 2):
                first, last = cp == 0, cp == NT - 2
                lhsT = h8[:, cp:cp + 2, ts(t, P)]
                nc.tensor.matmul(v_ps0[:], lhsT, sb_wv[:, cp:cp + 2, 0:512],
                                 start=first, stop=last, perf_mode=DR)
                nc.tensor.matmul(v_ps1[:], lhsT, sb_wv[:, cp:cp + 2, 512:1024],
                                 start=first, stop=last, perf_mode=DR)
            nc.vector.tensor_add(vT[:, t, 0:512], v_ps0[:], sb_bvb[:, 0:512])
            nc.vector.tensor_add(vT[:, t, 512:1024], v_ps1[:],
                                 sb_bvb[:, 512:1024])

        def k_proj_half(hp, h2):
            wk_t = work.tile([P, NT, P], F8, tag="W", bufs=5, name="wk_t")
            nc.sync.dma_start(out=wk_t[:], in_=d_wk[hp])
            k_ps = ps.tile([P, 512], F32, tag="bank", name="k_ps")
            for cp in range(0, NT, 2):
                nc.tensor.matmul(k_ps[:], wk_t[:, cp:cp + 2, :],
                                 h8[:, cp:cp + 2, ts(h2, 512)],
                                 start=(cp == 0), stop=(cp == NT - 2),
                                 perf_mode=DR)
            nc.vector.tensor_scalar(out=kT[:, hp, ts(h2, 512)], in0=k_ps[:],
                                    scalar1=sb_bk[:, hp:hp + 1], scalar2=None,
                                    op0=OP.add)

        s0 = ln1_stats(0)
        warm_pack(48, "wbr1")
        ln1_apply(0, *s0)
        for t in range(4):
            v_proj(t)
        for hp in range(NT):
            wq_t = work.tile([P, NT, P], F8, tag="W", bufs=5, name="wq_t")
            nc.sync.dma_start(out=wq_t[:], in_=d_wq[hp])
            q_ps = ps.tile([P, OWN], F32, tag="bank", name="q_ps")
            for cp in range(0, NT, 2):
                nc.tensor.matmul(q_ps[:], wq_t[:, cp:cp + 2, :],
                                 h8[:, cp:cp + 2, 0:OWN],
                                 start=(cp == 0), stop=(cp == NT - 2),
                                 perf_mode=DR)
            nc.vector.tensor_scalar(out=qT[:, hp, :], in0=q_ps[:],
                                    scalar1=sb_bq[:, hp:hp + 1], scalar2=None,
                                    op0=OP.add)
        s1 = ln1_stats(1)
        # K over the already-finished h2=0 tokens bridges LN1-h1's chain
        for hp in range(NT):
            k_proj_half(hp, 0)
        ln1_apply(1, *s1)
        for t in range(4, 8):
            v_proj(t)
        for hp in range(NT):
            k_proj_half(hp, 1)

        # ---------------- Phase 2: attention (software-pipelined) --------
        x2 = work.tile([P, NT, OWN], F32R, tag="A32", name="x2")
        CUM = [0, 128, 384, 768, 1280, 1408, 1664, 2048]

        def emit_scores(hp):
            ex_e = work.tile([P, 2560], BF, tag="exe", bufs=2, name="ex_e")
            ex_o = work.tile([P, 2560], BF, tag="exo", bufs=2, name="ex_o")
            for p in range(NT):
                w = NP_ACT[p] * P
                po = CUM[p]
                st_e = ps.tile([P, OWN], F32, tag="bank", name="st_e")
                st_o = ps.tile([P, OWN], F32, tag="bank", name="st_o")
                nc.tensor.matmul(st_e[:, 0:w], kT[0:64, hp, ts(p, P)],
                                 qT[0:64, hp, 0:w], start=True, stop=True)
                nc.tensor.matmul(st_o[:, 0:w], kT[64:128, hp, ts(p, P)],
                                 qT[64:128, hp, 0:w], start=True, stop=True)
                nc.scalar.activation(ex_e[:, po:po + w], st_e[:, 0:w], AF.Exp,
                                     scale=ATT_SCALE)
                nc.scalar.activation(ex_o[:, po:po + w], st_o[:, 0:w], AF.Exp,
                                     scale=ATT_SCALE)
                im = p % 4
                mw = slice(po + im * P, po + (im + 1) * P)
                nc.vector.tensor_mul(ex_e[:, mw], ex_e[:, mw], sb_mask[:, p, :])
                nc.vector.tensor_mul(ex_o[:, mw], ex_o[:, mw], sb_mask[:, p, :])
            return ex_e, ex_o

        def emit_av(hp, ex_e, ex_o):
            gat = work.tile([P, OWN], F32, tag="gat", bufs=1, name="gat")
            rs = work.tile([P, OWN], F32, tag="rs", bufs=1, name="rs")
            oTe = ps.tile([64, OWN], F32, tag="bank", name="oTe")
            oTo = ps.tile([P, OWN], F32, tag="bank", name="oTo")
            seE = ps.tile([64, OWN], F32, tag="bank", name="seE")
            seO = ps.tile([P, OWN], F32, tag="bank", name="seO")
            for p in [3, 0, 1, 2, 4, 5, 6, 7]:
                w = NP_ACT[p] * P
                first = (p == 3)
                last = (p == NT - 1)
                pc = slice(CUM[p], CUM[p] + w)
                nc.tensor.matmul(oTe[:, 0:w], vT[:, p, hp * P:hp * P + 64],
                                 ex_e[:, pc], start=first, stop=last)
                nc.tensor.matmul(oTo[64:128, 0:w],
                                 vT[:, p, hp * P + 64:hp * P + 128],
                                 ex_o[:, pc], start=first, stop=last)
                nc.tensor.matmul(seE[:, 0:w], s16_bf[:, :],
                                 ex_e[:, pc], start=first, stop=last)
                nc.tensor.matmul(seO[64:128, 0:w], s16_bf[:, :],
                                 ex_o[:, pc], start=first, stop=last)
            nc.vector.tensor_copy(gat[0:64, :], seE[:, :])
            nc.vector.tensor_copy(gat[64:128, :], seO[64:128, :])
            nc.vector.reciprocal_approx_fast(out=rs[:], in_=gat[:])
            ot = work.tile([P, OWN], F32, tag="ot", bufs=1, name="ot")
            nc.vector.tensor_mul(ot[0:64, :], oTe[:, :], rs[0:64, :])
            nc.vector.tensor_mul(ot[64:128, :], oTo[64:128, :], rs[64:128, :])
            nc.vector.tensor_add(x2[:, hp, :], ot[:, :], hbf[:, hp, 0:OWN])

        ex_cur = emit_scores(0)
        for hp in range(NT):
            ex_nxt = emit_scores(hp + 1) if hp + 1 < NT else None
            emit_av(hp, *ex_cur)
            ex_cur = ex_nxt

        # ------------- Phase 3: fused LN2+LNf, then FFN -------------
        hfb = work.tile([P, NT, OWN], BF, tag="C16", name="hfb")
        hf8 = work.tile([P, NT, OWN], F8, tag="D8", name="hf8")

        # fused stats over x2: six M=1 chains on distinct banks/col groups
        SA = ps.tile([P, 512], F32, tag="bank", name="SA")  # Sx    @0
        SB = ps.tile([P, 512], F32, tag="bank", name="SB")  # Sgx   @32
        SC = ps.tile([P, 512], F32, tag="bank", name="SC")  # Sg2x  @64
        SD = ps.tile([P, 512], F32, tag="bank", name="SD")  # Sgbx  @96
        SE = ps.tile([P, 512], F32, tag="bank", name="SE")  # Sx2   @0
        SF = ps.tile([P, 512], F32, tag="bank", name="SF")  # Sg2x2 @32
        for c in range(NT):
            sqf = work.tile([P, 512], BF, tag="sq", bufs=2, name="sqf")
            nc.vector.tensor_mul(sqf[:], x2[:, c, :], x2[:, c, :])
            st, sp = (c == 0), (c == NT - 1)
            xc = x2[:, c, :]
            nc.tensor.matmul(SA[0:1, :], ones_r[:, 0:1], xc, start=st, stop=sp)
            nc.tensor.matmul(SB[0:1, :], sb_statg[:, c:c + 1], xc,
                             start=st, stop=sp)
            nc.tensor.matmul(SC[0:1, :], sb_statg2[:, c:c + 1], xc,
                             start=st, stop=sp)
            nc.tensor.matmul(SD[0:1, :], sb_statgb[:, c:c + 1], xc,
                             start=st, stop=sp)
            nc.tensor.matmul(SE[0:1, :], ones_bf[:, 0:1], sqf[:],
                             start=st, stop=sp)
            nc.tensor.matmul(SF[64:65, :], sb_statq2[:, c:c + 1], sqf[:],
                             start=st, stop=sp)
        warm_pack(16, "wbr2")

        pfx = "l2f"
        mu, r = ln_core(SA[0:1, :], SE[0:1, :], 512, pfx)
        m2 = small(pfx, "t2")  # reuses t2-tag slot after ln_core
        nc.vector.tensor_mul(m2[:], mu[:], mu[:])
        # mup = r*(S1/C - gbar*mu) + bbar
        a1 = small(pfx, "a1")
        tt = small(pfx, "tt")
        mup = small(pfx, "mup")
        nc.scalar.activation(a1[:], SB[0:1, :], AF.Copy, scale=1.0 / C)
        nc.vector.tensor_scalar(out=tt[:], in0=mu[:], scalar1=sb_sc[0:1, 0:1],
                                scalar2=None, op0=OP.mult)
        nc.vector.tensor_sub(a1[:], a1[:], tt[:])
        nc.vector.tensor_mul(a1[:], a1[:], r[:])
        nc.vector.tensor_scalar(out=mup[:], in0=a1[:], scalar1=sb_sc[0:1, 1:2],
                                scalar2=None, op0=OP.add)
        # Ey2 = r^2*(Q1/C - mu*(2*S2/C) + m2*mg2) + r*(2*S3/C - mu*mgb2) + mb2
        q1 = small(pfx, "q1")
        a2 = small(pfx, "a1")  # a1 is dead once mup is computed
        nc.scalar.activation(q1[:], SF[64:65, :], AF.Copy, scale=1.0 / C)
        nc.scalar.activation(a2[:], SC[0:1, :], AF.Copy, scale=2.0 / C)
        nc.vector.tensor_mul(tt[:], a2[:], mu[:])
        nc.vector.tensor_sub(q1[:], q1[:], tt[:])
        nc.vector.tensor_scalar(out=tt[:], in0=m2[:], scalar1=sb_sc[0:1, 2:3],
                                scalar2=None, op0=OP.mult)
        nc.vector.tensor_add(q1[:], q1[:], tt[:])
        nc.vector.tensor_mul(q1[:], q1[:], r[:])
        nc.vector.tensor_mul(q1[:], q1[:], r[:])
        nc.scalar.activation(a2[:], SD[0:1, :], AF.Copy, scale=2.0 / C)
        nc.vector.tensor_scalar(out=tt[:], in0=mu[:], scalar1=sb_sc[0:1, 3:4],
                                scalar2=None, op0=OP.mult)
        nc.vector.tensor_sub(a2[:], a2[:], tt[:])
        nc.vector.tensor_mul(a2[:], a2[:], r[:])
        nc.vector.tensor_add(q1[:], q1[:], a2[:])
        nc.vector.tensor_scalar(out=q1[:], in0=q1[:], scalar1=sb_sc[0:1, 4:5],
                                scalar2=None, op0=OP.add)
        nc.vector.tensor_mul(tt[:], mup[:], mup[:])
        nc.vector.tensor_sub(q1[:], q1[:], tt[:])
        rp = small(pfx, "tt")  # tt slot is free by now
        nc.scalar.activation(q1[:], q1[:], AF.Sqrt, bias=eps1[:])
        nc.vector.reciprocal_approx_fast(out=rp[:], in_=q1[:])

        A2_bc, mB2 = bc_setup(mu, r, 512, "l2")
        Af_bc, mBf = bc_setup(mup, rp, 512, "lf", tagix=1)
        for c in range(NT):
            ln_apply(c, x2[:, c, :], x3[:, c, :], A2_bc, mB2, 1,
                     sb_g2[:, c:c + 1], 512, "l2")
            ln_apply(c, x3[:, c, :], hfb[:, c, :], Af_bc, mBf, 2,
                     sb_gf[:, c:c + 1], 512, "lf")
            nc.scalar.activation(hf8[:, c, :], hfb[:, c, :], AF.Copy)

        # ff1 (fp8 DoubleRow), relu output straight to fp8 with 1/256 fold
        relu8 = work.tile([P, 32, OWN], F8, tag="A32", name="relu8")
        for m in range(32):
            w1_t = work.tile([P, NT, P], F8, tag="W", bufs=5, name="w1_t")
            nc.sync.dma_start(out=w1_t[:], in_=d_w1[m])
            f_ps = ps.tile([P, OWN], F32, tag="bank", name="f_ps")
            for cp in range(0, NT, 2):
                nc.tensor.matmul(f_ps[:], w1_t[:, cp:cp + 2, :],
                                 hf8[:, cp:cp + 2, :],
                                 start=(cp == 0), stop=(cp == NT - 2),
                                 perf_mode=DR)
            nc.scalar.activation(relu8[:, m, :], f_ps[:], AF.Relu,
                                 bias=sb_b1f[:, m:m + 1], scale=1.0 / 256.0)

        # ff2 (fp8 DoubleRow) in two output-column groups for evac overlap
        for g in range(2):
            o_ps = [ps.tile([P, OWN], F32, tag="bank", name=f"o_ps{g}{j}")
                    for j in range(4)]
            for mb in range(16):
                w2t = work.tile([P, 2, 512], F8, tag="W", bufs=5, name="w2t")
                nc.sync.dma_start(out=w2t[:], in_=d_w2[g, mb])
                for jj in range(4):
                    nc.tensor.matmul(o_ps[jj][:], w2t[:, :, ts(jj, P)],
                                     relu8[:, 2 * mb:2 * mb + 2, :],
                                     start=(mb == 0), stop=(mb == 15),
                                     perf_mode=DR)
            for jj in range(4):
                co = 4 * g + jj
                u = work.tile([P, OWN], F32, tag="fft", bufs=2, name="fft")
                nc.scalar.activation(u[:], o_ps[jj][:], AF.Identity,
                                     bias=sb_b2f[:, co:co + 1],
                                     scale=1.0 / 256.0)
                nc.vector.tensor_add(u[:], u[:], x3[:, co, :])
                out_sb = work.tile([P, OWN], F32, tag="osb", bufs=2, name="osb")
                nc.vector.tensor_add(out_sb[:], u[:], hfb[:, co, :])
                nc.sync.dma_start(out=d_out[ts(co, P), :], in_=out_sb[:])

        ps.release()
        work.release()
        perst.release()
        const.release()

    nc.finalize()
    return nc


def _get_nc():
    if "nc" not in _CACHE:
        _CACHE["nc"] = _build_nc()
    return _CACHE["nc"]


def kernel(**inputs):
    from concourse.bass_utils import run_bass_kernel_spmd

    nc = _get_nc()
    e4 = ml_dtypes.float8_e4m3

    def q8(x):
        return np.clip(np.asarray(x, np.float32), -240.0, 240.0).astype(e4)

    f = {k: np.asarray(v, dtype=np.float32) for k, v in inputs.items()}
    x = f["x"]

    def stack_heads(w):  # [H, C, hd] -> [C, H*hd]
        return np.ascontiguousarray(w.transpose(1, 0, 2).reshape(C, C))

    def block_lhsT(w, nm):  # [C, nm*128] -> [nm, P, NT, P]
        return np.ascontiguousarray(
            w.reshape(NT, P, nm, P).transpose(2, 1, 0, 3))

    def part_scalar(v, n):  # [n*128] -> [128, n]
        return np.ascontiguousarray(v.reshape(-1).reshape(n, P).T)

    g1, b1 = f["g1"], f["b1"]
    g2, b2 = f["g2"], f["b2"]
    gf, bf = f["gf"], f["bf"]
    gbt = np.stack([np.stack([g1, b1]), np.stack([g2, b2]),
                    np.stack([gf, bf])], axis=0)  # [3, 2, C]
    gbt = gbt.transpose(1, 0, 2).reshape(2, 3, NT, P)
    sc = np.zeros((1, 8), np.float32)
    sc[0, 0] = g2.mean()
    sc[0, 1] = b2.mean()
    sc[0, 2] = (g2 * g2).mean()
    sc[0, 3] = 2.0 * (g2 * b2).mean()
    sc[0, 4] = (b2 * b2).mean()

    wq_full = stack_heads(f["Wq"]) * 16.0
    wk_full = stack_heads(f["Wk"]) * 16.0
    w2_grp = np.ascontiguousarray(
        (f["W2"] * 256.0).reshape(16, 2, P, 2, 512).transpose(3, 0, 2, 1, 4))
    shared = {
        "wq": q8(block_lhsT(wq_full, NT)),
        "wk": q8(block_lhsT(wk_full, NT)),
        "wv": q8(stack_heads(f["Wv"]) * 16.0),
        "w1": q8(block_lhsT(np.ascontiguousarray(f["W1"]) * 256.0, 32)),
        "w2": q8(w2_grp),
        "bq": part_scalar(f["bq"] * 16.0, NT),
        "bk": part_scalar(f["bk"] * 16.0, NT),
        "bvb": np.ascontiguousarray(
            np.broadcast_to(f["bv"].reshape(-1) * 16.0, (P, C))),
        "b1f": part_scalar(f["b1f"], 32),
        "b2f": part_scalar(f["b2f"], NT),
        "g1": part_scalar(g1, NT),
        "g2": part_scalar(g2, NT),
        "gf": part_scalar(gf, NT),
        "statg": part_scalar(g2, NT),
        "statg2": part_scalar(g2 * g2, NT),
        "statgb": part_scalar(g2 * b2, NT),
        "statq2": part_scalar(g2 * g2, NT).astype(ml_dtypes.bfloat16),
        "gbt": np.ascontiguousarray(gbt).astype(ml_dtypes.bfloat16),
        "sc": sc,
        "onesr": np.ones((P, 1), np.float32),
    }
    masks = {par: _mask01_for(par) for par in (0, 1)}
    perms = {par: _perm(par) for par in (0, 1)}

    in_maps = []
    for core in range(N_CORES):
        b, par = core // 2, core % 2
        xT = np.ascontiguousarray(x[b].T[:, perms[par]])
        in_maps.append({**shared, "xT": xT, "mask": masks[par]})

    res = run_bass_kernel_spmd(nc, in_maps, list(range(N_CORES)))

    out = np.empty((B, T, C), np.float32)
    for core in range(N_CORES):
        b, par = core // 2, core % 2
        outT = res.results[core]["outT"]  # [C, OWN]
        out[b, perms[par][:OWN], :] = outT.T
    return out
